# revision 25
# baseline (speedup 1.0000x reference)
"""AstrocyteGate distributed Bass kernel for one TRN2 chip (8 NeuronCores).

Reference computation (B=8, T=2048, D=2048, fp32):
    pooled    = mean over (B*T) of x            -> [D]
    update    = proj_w @ pooled + proj_b        -> [D]
    new_state = DECAY*state + (1-DECAY)*update  -> [D]
    gain      = sigmoid(gate_w @ new_state + gate_b)
    out       = x * gain                        (broadcast over [B,T,D])

Strategy (data-parallel over B, 1 batch row per core):
  - Each core DMA-loads its 16 MiB x shard into SBUF once and keeps it
    resident. A VectorE accumulation chain (acc += x_tile) runs behind the
    load DMAs.
  - The token-sum AllGather is split in two: the partial sum over the
    first 12 tiles gathers while the last 4 tiles still stream (this first
    collective also absorbs the ncfw init + rank rendezvous), then the
    remainder follows on a warm path. K=16 matmuls reduce the combined
    gather straight to pooled in partition layout ([128, 16]).
  - The two DxD matvecs are sharded: core c owns rows [256c, 256c+256) of
    proj_w and the matching columns of gate_w (bf16 shards, 1+1 MiB).
  - AllGather #2 combines the 8 partial gate logits; a single K=9
    all-ones-stationary matmul set both sums the ranks (incl. gate_b as a
    9th row) and broadcasts the logit across 128 partitions; sigmoid runs
    wide on [128, 2048]; the in-SBUF x tiles are scaled in place and
    streamed back out.

x is read from HBM exactly once and out written once (16+16 MiB per core,
plus 2 MiB of bf16 weight shards) -> memory roofline ~96us/core at 358 GB/s.
"""

import numpy as np

import concourse.bacc as bacc
import concourse.bass as bass
import concourse.mybir as mybir
import concourse.tile as tile
from concourse.bass_utils import run_bass_kernel_spmd

B, T, D = 8, 2048, 2048
NCORES = 8
DS = D // NCORES        # 256: per-core shard of D
MM = DS // 128          # 2:  128-row chunks per shard
TT = T // 128           # 16: token tiles per core / 128-col chunks of D
SPLIT = 12              # x tiles summed into the early (warm-up) gather
TAU = 1000.0
DECAY = float(np.exp(-1.0 / TAU))
FP32 = mybir.dt.float32
BF16 = mybir.dt.bfloat16
RG = [list(range(NCORES))]

_NC_CACHE = {}


def _build():
    nc = bacc.Bacc(
        "TRN2",
        target_bir_lowering=False,
        debug=False,
        enable_asserts=False,
        num_devices=NCORES,
    )

    x_d = nc.dram_tensor("x", [T, D], FP32, kind="ExternalInput")
    ptw_d = nc.dram_tensor("ptw", [128, TT, DS], BF16, kind="ExternalInput")
    gtw_d = nc.dram_tensor("gtw", [128, MM, D], BF16, kind="ExternalInput")
    st_d = nc.dram_tensor("st", [128, MM], FP32, kind="ExternalInput")
    pb_d = nc.dram_tensor("pb", [128, MM], FP32, kind="ExternalInput")
    gb_d = nc.dram_tensor("gb", [1, D], BF16, kind="ExternalInput")
    out_d = nc.dram_tensor("out", [T, D], FP32, kind="ExternalOutput")

    wsync_in = nc.dram_tensor("wsync_in", [1, 16], BF16)
    wsync_out = nc.dram_tensor("wsync_out", [NCORES, 16], BF16, addr_space="Shared")
    sum_bnc = nc.dram_tensor("sum_bnc", [128, TT], BF16)
    gath1 = nc.dram_tensor("gath1", [NCORES, 128, TT], BF16, addr_space="Shared")
    logit_bnc = nc.dram_tensor("logit_bnc", [1, D], BF16)
    gath2 = nc.dram_tensor("gath2", [NCORES, D], BF16, addr_space="Shared")

    AF = mybir.ActivationFunctionType
    ALU = mybir.AluOpType

    with tile.TileContext(nc) as tc:
        with (
            tc.tile_pool(name="xpool", bufs=TT) as xpool,
            tc.tile_pool(name="wpool", bufs=1) as wpool,
            tc.tile_pool(name="small", bufs=1) as small,
            tc.tile_pool(name="psA", bufs=1, space="PSUM") as psA,
            tc.tile_pool(name="psB", bufs=1, space="PSUM") as psB,
        ):
            # --- warm-up collective: ncfw wake + rank rendezvous, no deps ---
            nc.gpsimd.collective_compute(
                "AllGather",
                ALU.bypass,
                replica_groups=RG,
                ins=[wsync_in.ap().opt()],
                outs=[wsync_out.ap().opt()],
            )

            # --- load x first; everything else is off the critical path ---
            xs = []
            for j in range(TT):
                xt = xpool.tile([128, D], FP32, tag="xt")
                nc.sync.dma_start(xt[:], x_d[j * 128 : (j + 1) * 128, :])
                xs.append(xt)

            # --- constants ---
            ones1 = small.tile([128, 1], BF16, tag="ones1")
            nc.vector.memset(ones1[:], 1.0)
            invn = small.tile([NCORES, 1], BF16, tag="invn")
            nc.vector.memset(invn[:], 1.0 / float(B * T))
            ones9b = small.tile([NCORES + 1, 128], BF16, tag="ones9b")
            nc.vector.memset(ones9b[:], 1.0)
            # pre-warm the ScalarE sigmoid LUT off the critical path
            dummy = small.tile([1, 1], FP32, tag="dummy")
            nc.scalar.activation(dummy[:], ones1[0:1, 0:1], AF.Sigmoid)

            # --- weight / small-input loads ---
            ptw = wpool.tile([128, TT, DS], BF16, tag="ptw")
            nc.sync.dma_start(ptw[:], ptw_d[:])
            gtw = wpool.tile([128, MM, D], BF16, tag="gtw")
            nc.sync.dma_start(gtw[:], gtw_d[:])
            st = small.tile([128, MM], FP32, tag="st")
            nc.sync.dma_start(st[:], st_d[:])
            pb = small.tile([128, MM], FP32, tag="pb")
            nc.sync.dma_start(pb[:], pb_d[:])
            # gather tile for AG#2: rows 0..7 = gathered logits, row 8 = gate_b
            g2 = small.tile([NCORES + 1, D], BF16, tag="g2")
            nc.sync.dma_start(g2[NCORES : NCORES + 1, :], gb_d[:])

            # --- accumulate token-sums on VectorE as tiles land ---
            acc = wpool.tile([128, D], FP32, tag="acc")
            nc.vector.tensor_copy(acc[:], xs[0][:])
            for j in range(1, TT):
                nc.vector.tensor_add(acc[:], acc[:], xs[j][:])
            acc_bf = wpool.tile([128, D], BF16, tag="acc_bf")
            nc.scalar.copy(acc_bf[:], acc[:])
            # partition-reduce: sumT[p, j] = sum_p' acc[p', j*128+p]
            sumT_ps = psB.tile([128, TT], FP32, tag="pt")
            for j in range(TT):
                nc.tensor.matmul(
                    sumT_ps[:, j : j + 1],
                    acc_bf[:, j * 128 : (j + 1) * 128],
                    ones1[:],
                    start=True,
                    stop=True,
                )
            sumT = small.tile([128, TT], BF16, tag="sumT")
            nc.vector.tensor_copy(sumT[:], sumT_ps[:])
            nc.sync.dma_start(sum_bnc[:], sumT[:])
            nc.gpsimd.collective_compute(
                "AllGather",
                ALU.bypass,
                replica_groups=RG,
                ins=[sum_bnc.ap().opt()],
                outs=[gath1.ap().opt()],
            )

            # --- gather -> pooled [128, TT] via K=8 matmuls ---
            g1 = small.tile([NCORES, 128, TT], BF16, tag="g1")
            nc.sync.dma_start(g1[:], gath1[:])
            pooledT_ps = psB.tile([128, TT], FP32, tag="pt")
            for j in range(TT):
                nc.tensor.matmul(
                    pooledT_ps[:, j : j + 1],
                    g1[:, :, j],
                    invn[:],
                    start=True,
                    stop=True,
                )
            pooledT = small.tile([128, TT], BF16, tag="pooledT")
            nc.vector.tensor_copy(pooledT[:], pooledT_ps[:])

            # --- matvec1: update shard = proj_w[rows_c, :] @ pooled (bf16) ---
            upd_ps = psB.tile([128, MM], FP32, tag="upd")
            for m in range(MM):
                for kk in range(TT):
                    nc.tensor.matmul(
                        upd_ps[:, m : m + 1],
                        ptw[:, kk, m * 128 : (m + 1) * 128],
                        pooledT[:, kk : kk + 1],
                        start=(kk == 0),
                        stop=(kk == TT - 1),
                    )

            # --- EMA: ns = DECAY*state + (1-DECAY)*(update + proj_b) ---
            eb = small.tile([128, MM], FP32, tag="eb")
            nc.vector.tensor_scalar_mul(eb[:], pb[:], 1.0 - DECAY)
            nc.vector.scalar_tensor_tensor(eb[:], st[:], DECAY, eb[:], ALU.mult, ALU.add)
            ns = small.tile([128, MM], BF16, tag="ns")
            nc.vector.scalar_tensor_tensor(
                ns[:], upd_ps[:], 1.0 - DECAY, eb[:], ALU.mult, ALU.add
            )

            # --- matvec2: partial logit = gate_w[:, rows_c] @ ns_shard (bf16) ---
            logit_ps = psA.tile([1, D], FP32, tag="wide")
            for m in range(MM):
                for q in range(4):
                    nc.tensor.matmul(
                        logit_ps[0:1, q * 512 : (q + 1) * 512],
                        ns[:, m : m + 1],
                        gtw[:, m, q * 512 : (q + 1) * 512],
                        start=(m == 0),
                        stop=(m == MM - 1),
                    )
            logit = small.tile([1, D], BF16, tag="logit")
            nc.scalar.copy(logit[:], logit_ps[:])
            nc.sync.dma_start(logit_bnc[:], logit[:])

            # --- AllGather #2: partial logits (bf16) ---
            nc.gpsimd.collective_compute(
                "AllGather",
                ALU.bypass,
                replica_groups=RG,
                ins=[logit_bnc.ap().opt()],
                outs=[gath2.ap().opt()],
            )
            nc.sync.dma_start(g2[0:NCORES, :], gath2[:])

            # --- fused rank-sum + partition-broadcast of the gate logit:
            #     out[p, n] = sum_r g2[r, n]  (all-ones stationary, K=9) ---
            logit_bc_ps = psA.tile([128, D], FP32, tag="wide")
            for q in range(4):
                nc.tensor.matmul(
                    logit_bc_ps[:, q * 512 : (q + 1) * 512],
                    ones9b[:],
                    g2[:, q * 512 : (q + 1) * 512],
                    start=True,
                    stop=True,
                )
            gain_bc = wpool.tile([128, D], FP32, tag="gbc")
            nc.scalar.activation(gain_bc[:], logit_bc_ps[:], AF.Sigmoid)

            # --- scale x in place and stream out ---
            for j in range(TT):
                nc.vector.tensor_mul(xs[j][:], xs[j][:], gain_bc[:])
                nc.sync.dma_start(out_d[j * 128 : (j + 1) * 128, :], xs[j][:])

    nc.compile()
    return nc


def _get_nc():
    if "nc" not in _NC_CACHE:
        _NC_CACHE["nc"] = _build()
    return _NC_CACHE["nc"]


def _shard_inputs(x, state, proj_w, proj_b, gate_w, gate_b):
    import ml_dtypes

    bf16 = ml_dtypes.bfloat16
    x = np.asarray(x, dtype=np.float32)
    state = np.asarray(state, dtype=np.float32)
    proj_w = np.asarray(proj_w, dtype=np.float32)
    proj_b = np.asarray(proj_b, dtype=np.float32)
    gate_w = np.asarray(gate_w, dtype=np.float32)
    gate_b = np.asarray(gate_b, dtype=np.float32)

    gb = np.ascontiguousarray(gate_b.reshape(1, D).astype(bf16))
    in_maps = []
    for c in range(NCORES):
        lo, hi = c * DS, (c + 1) * DS
        # ptw[p, kk, m] = proj_w[lo + m, kk*128 + p]
        ptw = np.ascontiguousarray(
            proj_w[lo:hi, :].T.reshape(TT, 128, DS).transpose(1, 0, 2).astype(bf16)
        )
        # gtw[p, mm, n] = gate_w[n, lo + mm*128 + p]
        gtw = np.ascontiguousarray(
            gate_w[:, lo:hi].T.reshape(MM, 128, D).transpose(1, 0, 2).astype(bf16)
        )
        st = np.ascontiguousarray(state[lo:hi].reshape(MM, 128).T)
        pbc = np.ascontiguousarray(proj_b[lo:hi].reshape(MM, 128).T)
        in_maps.append(
            {
                "x": np.ascontiguousarray(x[c]),
                "ptw": ptw,
                "gtw": gtw,
                "st": st,
                "pb": pbc,
                "gb": gb,
            }
        )
    return in_maps


def _run(inputs, trace=False, **kwargs):
    nc = _get_nc()
    in_maps = _shard_inputs(**inputs)
    res = run_bass_kernel_spmd(
        nc, in_maps, core_ids=list(range(NCORES)), trace=trace, **kwargs
    )
    out = np.stack([res.results[c]["out"] for c in range(NCORES)], axis=0)
    return out, res


def kernel(**inputs):
    out, _ = _run(inputs, trace=False)
    return out


# revision 27
# speedup vs baseline: 1.0008x; 1.0008x over previous
"""AstrocyteGate distributed Bass kernel for one TRN2 chip (8 NeuronCores).

Reference computation (B=8, T=2048, D=2048, fp32):
    pooled    = mean over (B*T) of x            -> [D]
    update    = proj_w @ pooled + proj_b        -> [D]
    new_state = DECAY*state + (1-DECAY)*update  -> [D]
    gain      = sigmoid(gate_w @ new_state + gate_b)
    out       = x * gain                        (broadcast over [B,T,D])

Strategy (data-parallel over B, 1 batch row per core):
  - A zero-dependency warm-up AllGather issues first so the ncfw collective
    stack initializes and the 8 ranks rendezvous while the load DMAs stream
    (the first data-dependent collective then runs on a warm path).
  - Each core DMA-loads its 16 MiB x shard into SBUF once and keeps it
    resident. A VectorE accumulation chain (acc += x_tile) runs behind the
    load DMAs; acc is bf16-cast and partition-reduced by 16 small bf16
    matmuls into a [128, 16] token-sum (partition layout - no slow
    1-partition-wide ops on the critical path).
  - AllGather #1 combines the 8 local sums (bf16, 4 KB each); K=8 matmuls
    against a 1/(B*T) constant vector reduce the gather straight to pooled.
  - The two DxD matvecs are sharded: core c owns rows [256c, 256c+256) of
    proj_w and the matching columns of gate_w (bf16 shards, 1+1 MiB), as
    (LDWEIGHTS + N=1 MATMUL) chains the PE queue overlaps to ~33ns/pair.
  - AllGather #2 combines the 8 partial gate logits; a single K=9
    all-ones-stationary matmul set both sums the ranks (incl. gate_b as a
    9th row) and broadcasts the logit across 128 partitions; sigmoid runs
    wide on [128, 2048]; the in-SBUF x tiles are scaled in place and
    streamed back out.

x is read from HBM exactly once and out written once (16+16 MiB per core,
plus 2 MiB of bf16 weight shards) -> memory roofline ~96us/core at 358 GB/s.
"""

import numpy as np

import concourse.bacc as bacc
import concourse.bass as bass
import concourse.mybir as mybir
import concourse.tile as tile
from concourse.bass_utils import run_bass_kernel_spmd

B, T, D = 8, 2048, 2048
NCORES = 8
DS = D // NCORES        # 256: per-core shard of D
MM = DS // 128          # 2:  128-row chunks per shard
TT = T // 128           # 16: token tiles per core / 128-col chunks of D
TAU = 1000.0
DECAY = float(np.exp(-1.0 / TAU))
FP32 = mybir.dt.float32
BF16 = mybir.dt.bfloat16
RG = [list(range(NCORES))]

_NC_CACHE = {}


def _build():
    nc = bacc.Bacc(
        "TRN2",
        target_bir_lowering=False,
        debug=False,
        enable_asserts=False,
        num_devices=NCORES,
    )

    x_d = nc.dram_tensor("x", [T, D], FP32, kind="ExternalInput")
    ptw_d = nc.dram_tensor("ptw", [128, TT, DS], BF16, kind="ExternalInput")
    gtw_d = nc.dram_tensor("gtw", [128, MM, D], BF16, kind="ExternalInput")
    st_d = nc.dram_tensor("st", [128, MM], FP32, kind="ExternalInput")
    pb_d = nc.dram_tensor("pb", [128, MM], FP32, kind="ExternalInput")
    gb_d = nc.dram_tensor("gb", [1, D], BF16, kind="ExternalInput")
    out_d = nc.dram_tensor("out", [T, D], FP32, kind="ExternalOutput")

    wsync_in = nc.dram_tensor("wsync_in", [1, 16], BF16)
    wsync_out = nc.dram_tensor("wsync_out", [NCORES, 16], BF16, addr_space="Shared")
    sum_bnc = nc.dram_tensor("sum_bnc", [128, TT], BF16)
    gath1 = nc.dram_tensor("gath1", [NCORES, 128, TT], BF16, addr_space="Shared")
    logit_bnc = nc.dram_tensor("logit_bnc", [1, D], BF16)
    gath2 = nc.dram_tensor("gath2", [NCORES, D], BF16, addr_space="Shared")

    AF = mybir.ActivationFunctionType
    ALU = mybir.AluOpType

    with tile.TileContext(nc) as tc:
        with (
            tc.tile_pool(name="xpool", bufs=TT) as xpool,
            tc.tile_pool(name="wpool", bufs=1) as wpool,
            tc.tile_pool(name="small", bufs=1) as small,
            tc.tile_pool(name="psA", bufs=1, space="PSUM") as psA,
            tc.tile_pool(name="psB", bufs=1, space="PSUM") as psB,
        ):
            # --- warm-up collective: ncfw wake + rank rendezvous, no deps ---
            nc.gpsimd.collective_compute(
                "AllGather",
                ALU.bypass,
                replica_groups=RG,
                ins=[wsync_in.ap().opt()],
                outs=[wsync_out.ap().opt()],
            )

            # --- load x first; everything else is off the critical path ---
            xs = []
            for j in range(TT):
                xt = xpool.tile([128, D], FP32, tag="xt")
                nc.sync.dma_start(xt[:], x_d[j * 128 : (j + 1) * 128, :])
                xs.append(xt)

            # --- constants ---
            ones1 = small.tile([128, 1], BF16, tag="ones1")
            nc.vector.memset(ones1[:], 1.0)
            invn = small.tile([NCORES, 1], BF16, tag="invn")
            nc.vector.memset(invn[:], 1.0 / float(B * T))
            ones9b = small.tile([NCORES + 1, 128], BF16, tag="ones9b")
            nc.vector.memset(ones9b[:], 1.0)
            # pre-warm the ScalarE sigmoid LUT off the critical path
            dummy = small.tile([1, 1], FP32, tag="dummy")
            nc.scalar.activation(dummy[:], ones1[0:1, 0:1], AF.Sigmoid)

            # --- weight / small-input loads ---
            ptw = wpool.tile([128, TT, DS], BF16, tag="ptw")
            nc.sync.dma_start(ptw[:], ptw_d[:])
            gtw = wpool.tile([128, MM, D], BF16, tag="gtw")
            nc.sync.dma_start(gtw[:], gtw_d[:])
            st = small.tile([128, MM], FP32, tag="st")
            nc.sync.dma_start(st[:], st_d[:])
            pb = small.tile([128, MM], FP32, tag="pb")
            nc.sync.dma_start(pb[:], pb_d[:])
            # gather tile for AG#2: rows 0..7 = gathered logits, row 8 = gate_b
            g2 = small.tile([NCORES + 1, D], BF16, tag="g2")
            nc.sync.dma_start(g2[NCORES : NCORES + 1, :], gb_d[:])

            # --- accumulate token-sums on VectorE as tiles land ---
            acc = wpool.tile([128, D], FP32, tag="acc")
            nc.vector.tensor_copy(acc[:], xs[0][:])
            for j in range(1, TT):
                nc.vector.tensor_add(acc[:], acc[:], xs[j][:])
            acc_bf = wpool.tile([128, D], BF16, tag="acc_bf")
            nc.scalar.copy(acc_bf[:], acc[:])
            # partition-reduce: sumT[p, j] = sum_p' acc[p', j*128+p]
            sumT_ps = psB.tile([128, TT], FP32, tag="pt")
            for j in range(TT):
                nc.tensor.matmul(
                    sumT_ps[:, j : j + 1],
                    acc_bf[:, j * 128 : (j + 1) * 128],
                    ones1[:],
                    start=True,
                    stop=True,
                )
            sumT = small.tile([128, TT], BF16, tag="sumT")
            nc.vector.tensor_copy(sumT[:], sumT_ps[:])
            nc.sync.dma_start(sum_bnc[:], sumT[:])
            nc.gpsimd.collective_compute(
                "AllGather",
                ALU.bypass,
                replica_groups=RG,
                ins=[sum_bnc.ap().opt()],
                outs=[gath1.ap().opt()],
            )

            # --- gather -> pooled [128, TT] via K=8 matmuls ---
            g1 = small.tile([NCORES, 128, TT], BF16, tag="g1")
            nc.sync.dma_start(g1[:], gath1[:])
            pooledT_ps = psB.tile([128, TT], FP32, tag="pt")
            for j in range(TT):
                nc.tensor.matmul(
                    pooledT_ps[:, j : j + 1],
                    g1[:, :, j],
                    invn[:],
                    start=True,
                    stop=True,
                )
            pooledT = small.tile([128, TT], BF16, tag="pooledT")
            nc.vector.tensor_copy(pooledT[:], pooledT_ps[:])

            # --- matvec1: update shard = proj_w[rows_c, :] @ pooled (bf16) ---
            upd_ps = psB.tile([128, MM], FP32, tag="upd")
            for m in range(MM):
                for kk in range(TT):
                    nc.tensor.matmul(
                        upd_ps[:, m : m + 1],
                        ptw[:, kk, m * 128 : (m + 1) * 128],
                        pooledT[:, kk : kk + 1],
                        start=(kk == 0),
                        stop=(kk == TT - 1),
                    )

            # --- EMA: ns = DECAY*state + (1-DECAY)*(update + proj_b) ---
            eb = small.tile([128, MM], FP32, tag="eb")
            nc.vector.tensor_scalar_mul(eb[:], pb[:], 1.0 - DECAY)
            nc.vector.scalar_tensor_tensor(eb[:], st[:], DECAY, eb[:], ALU.mult, ALU.add)
            ns = small.tile([128, MM], BF16, tag="ns")
            nc.vector.scalar_tensor_tensor(
                ns[:], upd_ps[:], 1.0 - DECAY, eb[:], ALU.mult, ALU.add
            )

            # --- matvec2: partial logit = gate_w[:, rows_c] @ ns_shard (bf16) ---
            logit_ps = psA.tile([1, D], FP32, tag="wide")
            for m in range(MM):
                for q in range(4):
                    nc.tensor.matmul(
                        logit_ps[0:1, q * 512 : (q + 1) * 512],
                        ns[:, m : m + 1],
                        gtw[:, m, q * 512 : (q + 1) * 512],
                        start=(m == 0),
                        stop=(m == MM - 1),
                    )
            logit = small.tile([1, D], BF16, tag="logit")
            nc.scalar.copy(logit[:], logit_ps[:])
            nc.sync.dma_start(logit_bnc[:], logit[:])

            # --- AllGather #2: partial logits (bf16) ---
            nc.gpsimd.collective_compute(
                "AllGather",
                ALU.bypass,
                replica_groups=RG,
                ins=[logit_bnc.ap().opt()],
                outs=[gath2.ap().opt()],
            )
            nc.sync.dma_start(g2[0:NCORES, :], gath2[:])

            # --- fused rank-sum + partition-broadcast of the gate logit:
            #     out[p, n] = sum_r g2[r, n]  (all-ones stationary, K=9) ---
            logit_bc_ps = psA.tile([128, D], FP32, tag="wide")
            for q in range(4):
                nc.tensor.matmul(
                    logit_bc_ps[:, q * 512 : (q + 1) * 512],
                    ones9b[:],
                    g2[:, q * 512 : (q + 1) * 512],
                    start=True,
                    stop=True,
                )
            gain_bc = wpool.tile([128, D], FP32, tag="gbc")
            nc.scalar.activation(gain_bc[:], logit_bc_ps[:], AF.Sigmoid)

            # --- scale x in place and stream out ---
            for j in range(TT):
                nc.vector.tensor_mul(xs[j][:], xs[j][:], gain_bc[:])
                nc.sync.dma_start(out_d[j * 128 : (j + 1) * 128, :], xs[j][:])

    nc.compile()
    return nc


def _get_nc():
    if "nc" not in _NC_CACHE:
        _NC_CACHE["nc"] = _build()
    return _NC_CACHE["nc"]


def _shard_inputs(x, state, proj_w, proj_b, gate_w, gate_b):
    import ml_dtypes

    bf16 = ml_dtypes.bfloat16
    x = np.asarray(x, dtype=np.float32)
    state = np.asarray(state, dtype=np.float32)
    proj_w = np.asarray(proj_w, dtype=np.float32)
    proj_b = np.asarray(proj_b, dtype=np.float32)
    gate_w = np.asarray(gate_w, dtype=np.float32)
    gate_b = np.asarray(gate_b, dtype=np.float32)

    gb = np.ascontiguousarray(gate_b.reshape(1, D).astype(bf16))
    in_maps = []
    for c in range(NCORES):
        lo, hi = c * DS, (c + 1) * DS
        # ptw[p, kk, m] = proj_w[lo + m, kk*128 + p]
        ptw = np.ascontiguousarray(
            proj_w[lo:hi, :].T.reshape(TT, 128, DS).transpose(1, 0, 2).astype(bf16)
        )
        # gtw[p, mm, n] = gate_w[n, lo + mm*128 + p]
        gtw = np.ascontiguousarray(
            gate_w[:, lo:hi].T.reshape(MM, 128, D).transpose(1, 0, 2).astype(bf16)
        )
        st = np.ascontiguousarray(state[lo:hi].reshape(MM, 128).T)
        pbc = np.ascontiguousarray(proj_b[lo:hi].reshape(MM, 128).T)
        in_maps.append(
            {
                "x": np.ascontiguousarray(x[c]),
                "ptw": ptw,
                "gtw": gtw,
                "st": st,
                "pb": pbc,
                "gb": gb,
            }
        )
    return in_maps


def _run(inputs, trace=False, **kwargs):
    nc = _get_nc()
    in_maps = _shard_inputs(**inputs)
    res = run_bass_kernel_spmd(
        nc, in_maps, core_ids=list(range(NCORES)), trace=trace, **kwargs
    )
    out = np.stack([res.results[c]["out"] for c in range(NCORES)], axis=0)
    return out, res


def kernel(**inputs):
    out, _ = _run(inputs, trace=False)
    return out


# revision 31
# speedup vs baseline: 1.0140x; 1.0133x over previous
"""AstrocyteGate distributed Bass kernel for one TRN2 chip (8 NeuronCores).

Reference computation (B=8, T=2048, D=2048, fp32):
    pooled    = mean over (B*T) of x            -> [D]
    update    = proj_w @ pooled + proj_b        -> [D]
    new_state = DECAY*state + (1-DECAY)*update  -> [D]
    gain      = sigmoid(gate_w @ new_state + gate_b)
    out       = x * gain                        (broadcast over [B,T,D])

Strategy (data-parallel over B, 1 batch row per core):
  - A zero-dependency warm-up AllGather issues first so the ncfw collective
    stack initializes and the 8 ranks rendezvous while the load DMAs stream
    (the first data-dependent collective then runs on a warm path).
  - Each core DMA-loads its 16 MiB x shard into SBUF once and keeps it
    resident. A VectorE accumulation chain (acc += x_tile) runs behind the
    load DMAs; acc is bf16-cast and partition-reduced by 16 small bf16
    matmuls into a [128, 16] token-sum (partition layout - no slow
    1-partition-wide ops on the critical path).
  - AllGather #1 combines the 8 local sums (bf16, 4 KB each); K=8 matmuls
    against a 1/(B*T) constant vector reduce the gather straight to pooled.
  - The two DxD matvecs are sharded: core c owns rows [256c, 256c+256) of
    proj_w and the matching columns of gate_w (bf16 shards, 1+1 MiB), as
    (LDWEIGHTS + N=1 MATMUL) chains the PE queue overlaps to ~33ns/pair.
  - AllGather #2 combines the 8 partial gate logits; a single K=9
    all-ones-stationary matmul set both sums the ranks (incl. gate_b as a
    9th row) and broadcasts the logit across 128 partitions; sigmoid runs
    wide on [128, 2048]; the in-SBUF x tiles are scaled in place and
    streamed back out.

x is read from HBM exactly once and out written once (16+16 MiB per core,
plus 2 MiB of bf16 weight shards) -> memory roofline ~96us/core at 358 GB/s.
"""

import numpy as np

import concourse.bacc as bacc
import concourse.bass as bass
import concourse.mybir as mybir
import concourse.tile as tile
from concourse.bass_utils import run_bass_kernel_spmd

B, T, D = 8, 2048, 2048
NCORES = 8
DS = D // NCORES        # 256: per-core shard of D
MM = DS // 128          # 2:  128-row chunks per shard
TT = T // 128           # 16: token tiles per core / 128-col chunks of D
TAU = 1000.0
DECAY = float(np.exp(-1.0 / TAU))
FP32 = mybir.dt.float32
BF16 = mybir.dt.bfloat16
RG = [list(range(NCORES))]

_NC_CACHE = {}


def _build():
    nc = bacc.Bacc(
        "TRN2",
        target_bir_lowering=False,
        debug=False,
        enable_asserts=False,
        num_devices=NCORES,
    )

    x_d = nc.dram_tensor("x", [T, D], FP32, kind="ExternalInput")
    ptw_d = nc.dram_tensor("ptw", [128, TT, DS], BF16, kind="ExternalInput")
    gtw_d = nc.dram_tensor("gtw", [128, MM, D], BF16, kind="ExternalInput")
    st_d = nc.dram_tensor("st", [128, MM], FP32, kind="ExternalInput")
    pb_d = nc.dram_tensor("pb", [128, MM], FP32, kind="ExternalInput")
    gb_d = nc.dram_tensor("gb", [1, D], BF16, kind="ExternalInput")
    out_d = nc.dram_tensor("out", [T, D], FP32, kind="ExternalOutput")

    wsync_in = nc.dram_tensor("wsync_in", [1, 16], BF16)
    wsync_out = nc.dram_tensor("wsync_out", [NCORES, 16], BF16, addr_space="Shared")
    sum_bnc = nc.dram_tensor("sum_bnc", [128, TT], BF16)
    gath1 = nc.dram_tensor("gath1", [NCORES, 128, TT], BF16, addr_space="Shared")
    logit_bnc = nc.dram_tensor("logit_bnc", [1, D], BF16)
    gath2 = nc.dram_tensor("gath2", [NCORES, D], BF16, addr_space="Shared")

    AF = mybir.ActivationFunctionType
    ALU = mybir.AluOpType

    with tile.TileContext(nc) as tc:
        with (
            tc.tile_pool(name="xpool", bufs=TT) as xpool,
            tc.tile_pool(name="wpool", bufs=1) as wpool,
            tc.tile_pool(name="small", bufs=1) as small,
            tc.tile_pool(name="psA", bufs=1, space="PSUM") as psA,
            tc.tile_pool(name="psB", bufs=1, space="PSUM") as psB,
        ):
            # --- warm-up collective: ncfw wake + rank rendezvous, no deps ---
            nc.gpsimd.collective_compute(
                "AllGather",
                ALU.bypass,
                replica_groups=RG,
                ins=[wsync_in.ap().opt()],
                outs=[wsync_out.ap().opt()],
            )

            # --- load x first; everything else is off the critical path ---
            xs = []
            for j in range(TT):
                xt = xpool.tile([128, D], FP32, tag="xt")
                nc.sync.dma_start(xt[:], x_d[j * 128 : (j + 1) * 128, :])
                xs.append(xt)

            # --- constants ---
            ones1 = small.tile([128, 1], BF16, tag="ones1")
            nc.vector.memset(ones1[:], 1.0)
            invn = small.tile([NCORES, 1], BF16, tag="invn")
            nc.vector.memset(invn[:], 1.0 / float(B * T))
            ones9b = small.tile([NCORES + 1, 128], BF16, tag="ones9b")
            nc.vector.memset(ones9b[:], 1.0)
            # pre-warm the ScalarE sigmoid LUT off the critical path
            dummy = small.tile([1, 1], FP32, tag="dummy")
            nc.scalar.activation(dummy[:], ones1[0:1, 0:1], AF.Sigmoid)

            # --- weight / small-input loads ---
            ptw = wpool.tile([128, TT, DS], BF16, tag="ptw")
            nc.sync.dma_start(ptw[:], ptw_d[:])
            gtw = wpool.tile([128, MM, D], BF16, tag="gtw")
            nc.sync.dma_start(gtw[:], gtw_d[:])
            st = small.tile([128, MM], FP32, tag="st")
            nc.sync.dma_start(st[:], st_d[:])
            pb = small.tile([128, MM], FP32, tag="pb")
            nc.sync.dma_start(pb[:], pb_d[:])
            # gather tile for AG#2: rows 0..7 = gathered logits, row 8 = gate_b
            g2 = small.tile([NCORES + 1, D], BF16, tag="g2")
            nc.sync.dma_start(g2[NCORES : NCORES + 1, :], gb_d[:])

            # --- accumulate token-sums on VectorE as tiles land ---
            acc = wpool.tile([128, D], FP32, tag="acc")
            nc.vector.tensor_copy(acc[:], xs[0][:])
            for j in range(1, TT):
                nc.vector.tensor_add(acc[:], acc[:], xs[j][:])
            acc_bf = wpool.tile([128, D], BF16, tag="acc_bf")
            nc.scalar.copy(acc_bf[:], acc[:])
            # partition-reduce: sumT[p, j] = sum_p' acc[p', j*128+p]
            sumT_ps = psB.tile([128, TT], FP32, tag="pt")
            for j in range(TT):
                nc.tensor.matmul(
                    sumT_ps[:, j : j + 1],
                    acc_bf[:, j * 128 : (j + 1) * 128],
                    ones1[:],
                    start=True,
                    stop=True,
                )
            sumT = small.tile([128, TT], BF16, tag="sumT")
            nc.vector.tensor_copy(sumT[:], sumT_ps[:])
            nc.sync.dma_start(sum_bnc[:], sumT[:])
            nc.gpsimd.collective_compute(
                "AllGather",
                ALU.bypass,
                replica_groups=RG,
                ins=[sum_bnc.ap().opt()],
                outs=[gath1.ap().opt()],
            )

            # --- gather -> pooled [128, TT] via K=8 matmuls ---
            g1 = small.tile([NCORES, 128, TT], BF16, tag="g1")
            nc.sync.dma_start(g1[:], gath1[:])
            pooledT_ps = psB.tile([128, TT], FP32, tag="pt")
            for j in range(TT):
                nc.tensor.matmul(
                    pooledT_ps[:, j : j + 1],
                    g1[:, :, j],
                    invn[:],
                    start=True,
                    stop=True,
                )
            pooledT = small.tile([128, TT], BF16, tag="pooledT")
            nc.vector.tensor_copy(pooledT[:], pooledT_ps[:])

            # --- matvec1: update shard = proj_w[rows_c, :] @ pooled (bf16) ---
            upd_ps = psB.tile([128, MM], FP32, tag="upd")
            for m in range(MM):
                for kk in range(TT):
                    nc.tensor.matmul(
                        upd_ps[:, m : m + 1],
                        ptw[:, kk, m * 128 : (m + 1) * 128],
                        pooledT[:, kk : kk + 1],
                        start=(kk == 0),
                        stop=(kk == TT - 1),
                    )

            # --- EMA: ns = DECAY*state + (1-DECAY)*(update + proj_b) ---
            eb = small.tile([128, MM], FP32, tag="eb")
            nc.vector.tensor_scalar_mul(eb[:], pb[:], 1.0 - DECAY)
            nc.vector.scalar_tensor_tensor(eb[:], st[:], DECAY, eb[:], ALU.mult, ALU.add)
            ns = small.tile([128, MM], BF16, tag="ns")
            nc.vector.scalar_tensor_tensor(
                ns[:], upd_ps[:], 1.0 - DECAY, eb[:], ALU.mult, ALU.add
            )

            # --- matvec2: partial logit = gate_w[:, rows_c] @ ns_shard (bf16) ---
            logit_ps = psA.tile([1, D], FP32, tag="wide")
            for m in range(MM):
                for q in range(4):
                    nc.tensor.matmul(
                        logit_ps[0:1, q * 512 : (q + 1) * 512],
                        ns[:, m : m + 1],
                        gtw[:, m, q * 512 : (q + 1) * 512],
                        start=(m == 0),
                        stop=(m == MM - 1),
                    )
            logit = small.tile([1, D], BF16, tag="logit")
            nc.scalar.copy(logit[:], logit_ps[:])
            nc.sync.dma_start(logit_bnc[:], logit[:])

            # --- AllGather #2: partial logits (bf16) ---
            nc.gpsimd.collective_compute(
                "AllGather",
                ALU.bypass,
                replica_groups=RG,
                ins=[logit_bnc.ap().opt()],
                outs=[gath2.ap().opt()],
            )
            nc.sync.dma_start(g2[0:NCORES, :], gath2[:])

            # --- fused rank-sum + partition-broadcast of the gate logit:
            #     out[p, n] = sum_r g2[r, n]  (all-ones stationary, K=9) ---
            logit_bc_ps = psA.tile([128, D], FP32, tag="wide")
            for q in range(4):
                nc.tensor.matmul(
                    logit_bc_ps[:, q * 512 : (q + 1) * 512],
                    ones9b[:],
                    g2[:, q * 512 : (q + 1) * 512],
                    start=True,
                    stop=True,
                )
            gain_bc = wpool.tile([128, D], FP32, tag="gbc")
            nc.scalar.activation(gain_bc[:], logit_bc_ps[:], AF.Sigmoid)

            # --- scale x in place and stream out ---
            for j in range(TT):
                nc.vector.tensor_mul(xs[j][:], xs[j][:], gain_bc[:])
                nc.sync.dma_start(out_d[j * 128 : (j + 1) * 128, :], xs[j][:])

    nc.compile()
    return nc


def _get_nc():
    if "nc" not in _NC_CACHE:
        _NC_CACHE["nc"] = _build()
    return _NC_CACHE["nc"]


def _shard_inputs(x, state, proj_w, proj_b, gate_w, gate_b):
    import ml_dtypes

    bf16 = ml_dtypes.bfloat16
    x = np.asarray(x, dtype=np.float32)
    state = np.asarray(state, dtype=np.float32)
    proj_w = np.asarray(proj_w, dtype=np.float32)
    proj_b = np.asarray(proj_b, dtype=np.float32)
    gate_w = np.asarray(gate_w, dtype=np.float32)
    gate_b = np.asarray(gate_b, dtype=np.float32)

    gb = np.ascontiguousarray(gate_b.reshape(1, D).astype(bf16))
    in_maps = []
    for c in range(NCORES):
        lo, hi = c * DS, (c + 1) * DS
        # ptw[p, kk, m] = proj_w[lo + m, kk*128 + p]
        ptw = np.ascontiguousarray(
            proj_w[lo:hi, :].T.reshape(TT, 128, DS).transpose(1, 0, 2).astype(bf16)
        )
        # gtw[p, mm, n] = gate_w[n, lo + mm*128 + p]
        gtw = np.ascontiguousarray(
            gate_w[:, lo:hi].T.reshape(MM, 128, D).transpose(1, 0, 2).astype(bf16)
        )
        st = np.ascontiguousarray(state[lo:hi].reshape(MM, 128).T)
        pbc = np.ascontiguousarray(proj_b[lo:hi].reshape(MM, 128).T)
        in_maps.append(
            {
                "x": np.ascontiguousarray(x[c]),
                "ptw": ptw,
                "gtw": gtw,
                "st": st,
                "pb": pbc,
                "gb": gb,
            }
        )
    return in_maps


def _run(inputs, trace=False, **kwargs):
    nc = _get_nc()
    in_maps = _shard_inputs(**inputs)
    res = run_bass_kernel_spmd(
        nc, in_maps, core_ids=list(range(NCORES)), trace=trace, **kwargs
    )
    out = np.stack([res.results[c]["out"] for c in range(NCORES)], axis=0)
    return out, res


def kernel(**inputs):
    out, _ = _run(inputs, trace=False)
    return out
